# revision 17
# baseline (speedup 1.0000x reference)
"""GCN classifier (2x GCNConv + mean-pool + 2-layer MLP) on 8 Trainium2 cores.

Sharding strategy (graph/data parallel per the hint):
- Nodes partitioned contiguously: core c owns dst nodes [c*6250, (c+1)*6250).
- conv1 edges partitioned by dst owner, grouped into 49 windows of 128 dst
  nodes, padded to 128-edge chunks (uniform across cores -> one SPMD program).
- conv1 aggregation: host ships each core its incident edges' dinv[s]*x[s]
  rows (bf16, chunk-ordered -> pure sequential DMA streams) plus the pure
  0/1 one-hot scatter matrices in fp8 (exact values, 1 byte/entry);
  scatter-add realized as fp8xbf16 matmuls.
- Sym-norm factorization: out[d] = dinv[d] * sum_e dinv[s]*x[s].
- conv1 dense (W1) feature-major after PE transposes; h1 = relu(.) bf16;
  p = dinv * (h1 @ W2) node-major (carries conv2's source-side dinv).
- conv2 + mean-pool fusion: conv2's output is only consumed by the per-graph
  pool, so pool[g] = sum_edges dinv[d]*1[batch[d]==g] * p[s] = B @ p with a
  host-precomputed dense B [64, N] (self-loops folded in). Per core this is
  49 accumulating matmuls lhsT=B_chunk [128,64] x p_chunk [128,256] -> no
  halo exchange, no gathers. A ReduceScatter combines cores (8 graphs per
  core), each core runs the tiny MLP on its graphs, and an AllGather of
  [8,16] assembles the output; core 0's copy wins.
"""

import sys
import types

import ml_dtypes
import numpy as np

try:
    import antenv  # noqa: F401

    if "antenv.axon_hooks" not in sys.modules:
        _m = types.ModuleType("antenv.axon_hooks")
        _m._hook = None
        _m.set_axon_ntff_profile_hook = lambda h: setattr(_m, "_hook", h)
        _m.get_axon_ntff_profile_hook = lambda: _m._hook
        sys.modules["antenv.axon_hooks"] = _m
except Exception:
    pass

import concourse.bacc as bacc
import concourse.mybir as mybir
import concourse.tile as tile
from concourse import bass_utils
from concourse.masks import make_identity

F32 = mybir.dt.float32
BF16 = mybir.dt.bfloat16
FP8 = mybir.dt.float8e4
AF = mybir.ActivationFunctionType
OP = mybir.AluOpType

N = 50000
E = 500000
DIN = 256
DH = 512
NG = 64
DOUT = 16

NCORES = 8
SLICE = N // NCORES  # 6250
NW = (SLICE + 127) // 128  # 49 windows
NPAD = NW * 128  # 6272
GB = 2  # windows per batch
NB = (NW + GB - 1) // GB  # 25
NGRP = (NPAD + 511) // 512  # 13 dense groups of 512 nodes

_COMPILED: dict = {}


def _layout1(K1):
    """conv1 layout: per batch [w0 chunks | w1 chunks]. Returns batches, total."""
    batches = []
    gcol = 0
    for b in range(NB):
        ws = list(range(b * GB, min(NW, b * GB + GB)))
        wchunks = {w: [] for w in ws}
        rel = 0
        for w in ws:
            for _ in range(int(K1[w])):
                wchunks[w].append((gcol, rel))
                gcol += 1
                rel += 1
        batches.append((ws, wchunks, rel))
    return batches, gcol


def _preprocess(x, edge_index, batch):
    src = np.asarray(edge_index[0], dtype=np.int64)
    dst = np.asarray(edge_index[1], dtype=np.int64)
    batch = np.asarray(batch, dtype=np.int64)

    deg = np.bincount(dst, minlength=N).astype(np.float64) + 1.0
    dinv = (1.0 / np.sqrt(deg)).astype(np.float32)
    cnt = np.maximum(np.bincount(batch, minlength=NG), 1)

    loops = np.arange(N, dtype=np.int64)

    # ---------- conv1: edges + self-loops, grouped by (core, window) ----------
    s1 = np.concatenate([src, loops])
    d1 = np.concatenate([dst, loops])
    core1 = d1 // SLICE
    win1 = (d1 % SLICE) // 128
    key1 = core1 * NW + win1
    order1 = np.argsort(key1, kind="stable")
    ss1, ds1 = s1[order1], d1[order1]
    counts1 = np.bincount(key1, minlength=NCORES * NW).reshape(NCORES, NW)
    starts1 = np.zeros(NCORES * NW + 1, dtype=np.int64)
    np.cumsum(counts1.reshape(-1), out=starts1[1:])
    K1 = np.ceil(counts1.max(axis=0) / 128).astype(np.int64)  # [NW]

    meta = tuple(int(v) for v in K1)
    b1, C1 = _layout1(K1)

    # x rows pre-scaled by source-side dinv -> one-hots stay pure 0/1
    xs = (np.asarray(x, np.float32) * dinv[:, None]).astype(ml_dtypes.bfloat16)

    # ---------- conv2+pool fused: B[g, s] = sum_{(s,d)} dinv[d]*[batch[d]==g] ----
    B = np.zeros((NG, N), dtype=np.float32)
    np.add.at(B, (batch[dst], src), dinv[dst])
    B[batch, loops] += dinv  # self-loops

    per_core = []
    for c in range(NCORES):
        # conv1 arrays
        src_cols = np.zeros((C1, 128), dtype=np.int64)
        dst1_cols = np.full((C1, 128), -1.0, dtype=np.float32)
        for ws, wchunks, _rel in b1:
            for w in ws:
                gi = c * NW + w
                e0, e1 = starts1[gi], starts1[gi + 1]
                n_e = int(e1 - e0)
                cols = wchunks[w]
                k = len(cols)
                sv = np.zeros(k * 128, dtype=np.int64)
                sv[:n_e] = ss1[e0:e1]
                dv = np.full(k * 128, -1.0, dtype=np.float32)
                dv[:n_e] = (ds1[e0:e1] - (c * SLICE + w * 128)).astype(np.float32)
                for j, (gcol, _r) in enumerate(cols):
                    src_cols[gcol] = sv[j * 128 : (j + 1) * 128]
                    dst1_cols[gcol] = dv[j * 128 : (j + 1) * 128]
        x_edges = np.ascontiguousarray(
            xs[src_cols.reshape(-1)].reshape(C1, 128, DIN).transpose(1, 0, 2)
        )  # [128, C1, DIN] bf16
        oh = (
            (dst1_cols[:, :, None] == np.arange(128, dtype=np.float32)[None, None, :])
            .astype(ml_dtypes.float8_e4m3)
            .transpose(1, 0, 2)
        )  # [128, C1, 128] fp8, pure 0/1

        nodes = np.arange(c * SLICE, (c + 1) * SLICE)
        tmp = np.ones(NPAD, dtype=np.float32)
        tmp[:SLICE] = dinv[nodes]
        dinv_col = np.ascontiguousarray(tmp.reshape(NW, 128).T)  # [128, NW] f32

        Bc = np.zeros((NPAD, NG), dtype=np.float32)
        Bc[:SLICE] = B[:, c * SLICE : (c + 1) * SLICE].T
        Bt = np.ascontiguousarray(
            Bc.reshape(NW, 128, NG).transpose(1, 0, 2).reshape(128, NW * NG)
        ).astype(ml_dtypes.bfloat16)  # [128, NW*64]

        per_core.append(
            dict(
                x_edges=x_edges.reshape(128, C1 * DIN),
                oh=np.ascontiguousarray(oh.reshape(128, C1 * 128)),
                dinv_col=dinv_col,
                Bt=Bt,
                cnt=cnt[c * (NG // NCORES) : (c + 1) * (NG // NCORES)]
                .astype(np.float32)
                .reshape(NG // NCORES, 1),
            )
        )
    return meta, per_core


def _build_program(meta):
    K1 = np.array(meta)
    b1, C1 = _layout1(K1)

    nc = bacc.Bacc("TRN2", target_bir_lowering=False, debug=False, num_devices=NCORES)

    def din(name, shape, dt=F32):
        return nc.dram_tensor(name, shape, dt, kind="ExternalInput").ap()

    NGC = NG // NCORES  # graphs per core after ReduceScatter
    x_edges = din("x_edges", [128, C1 * DIN], BF16)
    oh_in = din("oh", [128, C1 * 128], FP8)
    dinv_col_in = din("dinv_col", [128, NW])
    Bt_in = din("Bt", [128, NW * NG], BF16)
    cnt_in = din("cnt", [NGC, 1])
    W1t = din("W1t", [128, 2 * DH], BF16)
    b1c = din("b1c", [128, DH // 128])
    W2t = din("W2t", [128, 4 * (DH // 2)], BF16)
    b2r = din("b2r", [128, DH // 2])
    Wf1 = din("Wf1", [DH // 2, DH // 4])
    bf1c = din("bf1c", [128, 1])
    Wf2 = din("Wf2", [DH // 4, DOUT])
    bf2c = din("bf2c", [DOUT, 1])
    out = nc.dram_tensor("out", [NG, DOUT], F32, kind="ExternalOutput").ap()

    with tile.TileContext(nc) as tc:
        with (
            tc.tile_pool(name="const", bufs=1) as cp,
            tc.tile_pool(name="big", bufs=1) as bigp,
            tc.tile_pool(name="work", bufs=1) as wp,
            tc.tile_pool(name="psum", bufs=1, space="PSUM") as pp,
            tc.tile_pool(name="dram", bufs=1, space="DRAM") as dp,
        ):
            def load(ap_in, shape, dt=F32, pool=cp):
                t = pool.tile(shape, dt, name=ap_in.tensor.name + "_sb")
                nc.sync.dma_start(t[:], ap_in[:])
                return t

            dinv_col = load(dinv_col_in, [128, NW])
            Bt_sb = load(Bt_in, [128, NW * NG], BF16)
            W1_sb = load(W1t, [128, 2 * DH], BF16)
            W2_sb = load(W2t, [128, 4 * (DH // 2)], BF16)
            b1_sb = load(b1c, [128, DH // 128])
            b2_sb = load(b2r, [128, DH // 2])
            bf1_sb = load(bf1c, [128, 1])
            bf2_sb = load(bf2c, [DOUT, 1])
            cnt_sb = load(cnt_in, [NGC, 1])
            Wf1_sb = [cp.tile([128, DH // 4], F32, name=f"wf1_{k}") for k in range(2)]
            for k in range(2):
                nc.sync.dma_start(Wf1_sb[k][:], Wf1[k * 128 : (k + 1) * 128, :])
            Wf2_sb = cp.tile([128, DOUT], F32)
            nc.sync.dma_start(Wf2_sb[:], Wf2[:])
            idbf = cp.tile([128, 128], BF16)
            make_identity(nc, idbf[:])
            idf32 = cp.tile([128, 128], F32)
            make_identity(nc, idf32[:])

            h1s = [bigp.tile([128, NPAD], BF16, name=f"h1s_{k}") for k in range(4)]
            sfm_groups: dict = {}

            def sfm_of(g):
                if g not in sfm_groups:
                    sfm_groups[g] = [
                        wp.tile([128, 512], BF16, tag=f"sfm{k}", bufs=2, name=f"sfm{k}_{g}")
                        for k in range(2)
                    ]
                return sfm_groups[g]

            g_local = dp.tile([NG, DH // 2], F32)
            g_rs = dp.tile([NGC, DH // 2], F32)
            g_out_loc = dp.tile([NGC, DOUT], F32)
            g_out = dp.tile([NG, DOUT], F32, addr_space="Shared")
            pg = pp.tile([NG, DH // 2], F32, tag="pool", bufs=1, name="pg")

            # ---- phase A: conv1 aggregation (one batch = GB windows) ----
            def emit_agg_batch(ws, wchunks, nch):
                c0 = wchunks[ws[0]][0][0]
                G1 = wp.tile([128, nch, DIN], BF16, tag="G1", bufs=2, name=f"g1_{ws[0]}")
                nc.sync.dma_start(
                    G1[:].rearrange("p c d -> p (c d)"),
                    x_edges[:, c0 * DIN : (c0 + nch) * DIN],
                )
                oh1 = wp.tile([128, nch, 128], FP8, tag="oh", bufs=2, name=f"oh1_{ws[0]}")
                nc.sync.dma_start(
                    oh1[:].rearrange("p c d -> p (c d)"),
                    oh_in[:, c0 * 128 : (c0 + nch) * 128],
                )
                for w in ws:
                    cols = wchunks[w]
                    acc = pp.tile([128, DIN], F32, tag="agg", bufs=3, name=f"acc1_{w}")
                    for j, (gcol, grel) in enumerate(cols):
                        nc.tensor.matmul(
                            out=acc[:],
                            lhsT=oh1[:, grel, :],
                            rhs=G1[:, grel, :],
                            start=(j == 0),
                            stop=(j == len(cols) - 1),
                        )
                    snm = wp.tile([128, DIN], BF16, tag="snm", bufs=2, name=f"snm_{w}")
                    nc.scalar.activation(snm[:], acc[:], AF.Copy, scale=dinv_col[:, w : w + 1])
                    sf = sfm_of(w // 4)
                    wc = (w % 4) * 128
                    for k in range(2):
                        pt = pp.tile([128, 128], BF16, tag="t", bufs=2, name=f"pt_{w}_{k}")
                        nc.tensor.transpose(pt[:], snm[:, k * 128 : (k + 1) * 128], idbf[:])
                        if k == 0:
                            nc.scalar.activation(sf[k][:, wc : wc + 128], pt[:], AF.Copy)
                        else:
                            nc.vector.tensor_copy(sf[k][:, wc : wc + 128], pt[:])

            # ---- phase B: dense W1 (feature-major), relu -> h1s ----
            def emit_dense_group(g):
                c0 = g * 512
                cw = min(512, NPAD - c0)
                sf = sfm_of(g)
                for m in range(4):
                    ph = pp.tile([128, 512], F32, tag="h1", bufs=2, name=f"ph1_{g}_{m}")
                    for k in range(2):
                        nc.tensor.matmul(
                            out=ph[:, :cw],
                            lhsT=W1_sb[:, k * DH + m * 128 : k * DH + (m + 1) * 128],
                            rhs=sf[k][:, :cw],
                            start=(k == 0),
                            stop=(k == 1),
                        )
                    nc.scalar.activation(
                        h1s[m][:, c0 : c0 + cw], ph[:, :cw], AF.Relu, bias=b1_sb[:, m : m + 1]
                    )

            # ---- phase C: p = dinv*(h1@W2) node-major; fused pool matmul ----
            def emit_p_chunk(cc):
                c0 = cc * 128
                ppm = pp.tile([128, DH // 2], F32, tag="agg", bufs=3, name=f"pp_{cc}")
                for k in range(4):
                    nc.tensor.matmul(
                        out=ppm[:],
                        lhsT=h1s[k][:, c0 : c0 + 128],
                        rhs=W2_sb[:, k * (DH // 2) : (k + 1) * (DH // 2)],
                        start=(k == 0),
                        stop=(k == 3),
                    )
                pb = wp.tile([128, DH // 2], BF16, tag="pb", bufs=2, name=f"pb_{cc}")
                nc.vector.tensor_scalar_mul(pb[:], ppm[:], dinv_col[:, cc : cc + 1])
                nc.tensor.matmul(
                    out=pg[:],
                    lhsT=Bt_sb[:, cc * NG : (cc + 1) * NG],
                    rhs=pb[:],
                    start=(cc == 0),
                    stop=(cc == NW - 1),
                )

            g_done = 0
            p_done = 0
            for b in range(NB):
                emit_agg_batch(*b1[b])
                wins_done = min((b + 1) * GB, NW)
                while g_done < NGRP and (
                    (g_done + 1) * 4 <= wins_done or wins_done == NW
                ):
                    emit_dense_group(g_done)
                    g_done += 1
                    hi = min(g_done * 4, NW)
                    while p_done < hi:
                        emit_p_chunk(p_done)
                        p_done += 1

            # ---- phase E: ReduceScatter (8 graphs/core) + mean + relu ----
            gsb = wp.tile([NG, DH // 2], F32)
            nc.vector.tensor_copy(gsb[:], pg[:])
            nc.sync.dma_start(g_local[:], gsb[:])
            nc.gpsimd.collective_compute(
                "ReduceScatter",
                OP.add,
                replica_groups=[list(range(NCORES))],
                ins=[g_local.opt()],
                outs=[g_rs.opt()],
            )
            gsum = wp.tile([NGC, DH // 2], F32)
            nc.sync.dma_start(gsum[:], g_rs[:])
            cinv = wp.tile([NGC, 1], F32)
            nc.vector.reciprocal(cinv[:], cnt_sb[:])
            gmean = wp.tile([NGC, DH // 2], F32)
            nc.vector.scalar_tensor_tensor(
                out=gmean[:],
                in0=gsum[:],
                scalar=cinv[:, 0:1],
                in1=b2_sb[:NGC, :],
                op0=OP.mult,
                op1=OP.add,
            )
            grelu = wp.tile([NGC, DH // 2], F32)
            nc.scalar.activation(grelu[:], gmean[:], AF.Relu)

            # ---- phase F: MLP on this core's 8 graphs, then AllGather ----
            g_fm = [wp.tile([128, NGC], F32, name=f"gfm_{k}") for k in range(2)]
            for k in range(2):
                pt = pp.tile([128, NGC], F32, tag="t", bufs=2, name=f"gt_{k}")
                nc.tensor.transpose(pt[:], grelu[:, k * 128 : (k + 1) * 128], idf32[:NGC, :NGC])
                nc.vector.tensor_copy(g_fm[k][:], pt[:])
            pz = pp.tile([128, NGC], F32, tag="h1", bufs=2, name="pz")
            for k in range(2):
                nc.tensor.matmul(
                    out=pz[:], lhsT=Wf1_sb[k][:], rhs=g_fm[k][:], start=(k == 0), stop=(k == 1)
                )
            zsb = wp.tile([128, NGC], F32)
            nc.scalar.activation(zsb[:], pz[:], AF.Relu, bias=bf1_sb[:, 0:1])
            po = pp.tile([DOUT, NGC], F32, tag="t", bufs=2, name="po")
            nc.tensor.matmul(out=po[:], lhsT=Wf2_sb[:], rhs=zsb[:], start=True, stop=True)
            osb = wp.tile([DOUT, NGC], F32)
            nc.scalar.activation(osb[:], po[:], AF.Relu, bias=bf2_sb[:, 0:1])
            pout = pp.tile([NGC, DOUT], F32, tag="t", bufs=2, name="pout")
            nc.tensor.transpose(pout[:], osb[:], idf32[:DOUT, :DOUT])
            out_sb = wp.tile([NGC, DOUT], F32)
            nc.vector.tensor_copy(out_sb[:], pout[:])
            nc.sync.dma_start(g_out_loc[:], out_sb[:])
            nc.gpsimd.collective_compute(
                "AllGather",
                OP.bypass,
                replica_groups=[list(range(NCORES))],
                ins=[g_out_loc.opt()],
                outs=[g_out.opt()],
            )
            nc.sync.dma_start(out[:], g_out[:])

    nc.compile()
    return nc


def _get_program(meta):
    if meta not in _COMPILED:
        _COMPILED[meta] = _build_program(meta)
    return _COMPILED[meta]


def _make_in_maps(W1, b1, W2, b2, Wf1, bf1, Wf2, bf2, per_core):
    W1t = np.concatenate(
        [np.asarray(W1, np.float32)[k * 128 : (k + 1) * 128, :] for k in range(2)], axis=1
    ).astype(ml_dtypes.bfloat16)
    W2t = np.concatenate(
        [np.asarray(W2, np.float32)[k * 128 : (k + 1) * 128, :] for k in range(4)], axis=1
    ).astype(ml_dtypes.bfloat16)
    shared = dict(
        W1t=W1t,
        b1c=np.ascontiguousarray(np.asarray(b1, np.float32).reshape(DH // 128, 128).T),
        W2t=W2t,
        b2r=np.ascontiguousarray(np.tile(np.asarray(b2, np.float32)[None, :], (128, 1))),
        Wf1=np.asarray(Wf1, np.float32),
        bf1c=np.asarray(bf1, np.float32).reshape(DH // 4, 1),
        Wf2=np.asarray(Wf2, np.float32),
        bf2c=np.asarray(bf2, np.float32).reshape(DOUT, 1),
    )
    return [dict(shared, **per_core[c]) for c in range(NCORES)]


def kernel(
    x, W1, b1, W2, b2, Wf1, bf1, Wf2, bf2, edge_index, batch, num_graphs, _trace=False
):
    assert int(num_graphs) == NG
    meta, per_core = _preprocess(
        np.asarray(x), np.asarray(edge_index), np.asarray(batch)
    )
    nc = _get_program(meta)
    in_maps = _make_in_maps(W1, b1, W2, b2, Wf1, bf1, Wf2, bf2, per_core)
    res = bass_utils.run_bass_kernel_spmd(
        nc, in_maps, core_ids=list(range(NCORES)), trace=_trace
    )
    out = np.asarray(res.results[0]["out"], np.float32)
    if _trace:
        kernel._last_results = res
    return out


# revision 27
# speedup vs baseline: 1.0739x; 1.0739x over previous
"""GCN classifier (2x GCNConv + mean-pool + 2-layer MLP) on 8 Trainium2 cores.

Sharding strategy (graph/data parallel per the hint):
- Nodes partitioned contiguously: core c owns dst nodes [c*6250, (c+1)*6250).
- conv1 edges partitioned by dst owner, grouped into 49 windows of 128 dst
  nodes, padded to 128-edge chunks (uniform across cores -> one SPMD program).
- conv1 aggregation: host ships each core its incident edges' dinv[s]*x[s]
  rows (bf16, chunk-ordered -> pure sequential DMA streams) plus the pure
  0/1 one-hot scatter matrices in fp8 (exact values, 1 byte/entry);
  scatter-add realized as fp8xbf16 matmuls.
- Sym-norm factorization: out[d] = dinv[d] * sum_e dinv[s]*x[s].
- conv1 dense (W1) feature-major after PE transposes; h1 = relu(.) bf16;
  p = dinv * (h1 @ W2) node-major (carries conv2's source-side dinv).
- conv2 + mean-pool fusion: conv2's output is only consumed by the per-graph
  pool, so pool[g] = sum_edges dinv[d]*1[batch[d]==g] * p[s] = B @ p with a
  host-precomputed dense B [64, N] (self-loops folded in). Per core this is
  49 accumulating matmuls lhsT=B_chunk [128,64] x p_chunk [128,256] -> no
  halo exchange, no gathers. A ReduceScatter combines cores (8 graphs per
  core), each core runs the tiny MLP on its graphs, and an AllGather of
  [8,16] assembles the output; core 0's copy wins.
"""

import sys
import types

import ml_dtypes
import numpy as np

try:
    import antenv  # noqa: F401

    if "antenv.axon_hooks" not in sys.modules:
        _m = types.ModuleType("antenv.axon_hooks")
        _m._hook = None
        _m.set_axon_ntff_profile_hook = lambda h: setattr(_m, "_hook", h)
        _m.get_axon_ntff_profile_hook = lambda: _m._hook
        sys.modules["antenv.axon_hooks"] = _m
except Exception:
    pass

import concourse.bacc as bacc
import concourse.mybir as mybir
import concourse.tile as tile
from concourse import bass_utils
from concourse.masks import make_identity

F32 = mybir.dt.float32
BF16 = mybir.dt.bfloat16
FP8 = mybir.dt.float8e4
AF = mybir.ActivationFunctionType
OP = mybir.AluOpType

N = 50000
E = 500000
DIN = 256
DH = 512
NG = 64
DOUT = 16

NCORES = 8
SLICE = N // NCORES  # 6250
NW = (SLICE + 127) // 128  # 49 windows
NPAD = NW * 128  # 6272
GB = 2  # windows per batch
NB = (NW + GB - 1) // GB  # 25
NGRP = (NPAD + 511) // 512  # 13 dense groups of 512 nodes

_COMPILED: dict = {}


def _layout1(K1):
    """conv1 layout: per batch [w0 chunks | w1 chunks]. Returns batches, total."""
    batches = []
    gcol = 0
    for b in range(NB):
        ws = list(range(b * GB, min(NW, b * GB + GB)))
        wchunks = {w: [] for w in ws}
        rel = 0
        for w in ws:
            for _ in range(int(K1[w])):
                wchunks[w].append((gcol, rel))
                gcol += 1
                rel += 1
        batches.append((ws, wchunks, rel))
    return batches, gcol


def _preprocess(x, edge_index, batch):
    src = np.asarray(edge_index[0], dtype=np.int64)
    dst = np.asarray(edge_index[1], dtype=np.int64)
    batch = np.asarray(batch, dtype=np.int64)

    deg = np.bincount(dst, minlength=N).astype(np.float64) + 1.0
    dinv = (1.0 / np.sqrt(deg)).astype(np.float32)
    cnt = np.maximum(np.bincount(batch, minlength=NG), 1)

    loops = np.arange(N, dtype=np.int64)

    # ---------- conv1: edges + self-loops, grouped by (core, window) ----------
    s1 = np.concatenate([src, loops])
    d1 = np.concatenate([dst, loops])
    core1 = d1 // SLICE
    win1 = (d1 % SLICE) // 128
    key1 = core1 * NW + win1
    order1 = np.argsort(key1, kind="stable")
    ss1, ds1 = s1[order1], d1[order1]
    counts1 = np.bincount(key1, minlength=NCORES * NW).reshape(NCORES, NW)
    starts1 = np.zeros(NCORES * NW + 1, dtype=np.int64)
    np.cumsum(counts1.reshape(-1), out=starts1[1:])
    K1 = np.ceil(counts1.max(axis=0) / 128).astype(np.int64)  # [NW]

    meta = tuple(int(v) for v in K1)
    b1, C1 = _layout1(K1)

    # x rows pre-scaled by source-side dinv -> one-hots stay pure 0/1
    xs = (np.asarray(x, np.float32) * dinv[:, None]).astype(ml_dtypes.float8_e4m3)

    # ---------- conv2+pool fused: B[g, s] = sum_{(s,d)} dinv[d]*[batch[d]==g] ----
    B = np.zeros((NG, N), dtype=np.float32)
    np.add.at(B, (batch[dst], src), dinv[dst])
    B[batch, loops] += dinv  # self-loops

    per_core = []
    for c in range(NCORES):
        # conv1 arrays
        src_cols = np.zeros((C1, 128), dtype=np.int64)
        dst1_cols = np.full((C1, 128), -1.0, dtype=np.float32)
        for ws, wchunks, _rel in b1:
            for w in ws:
                gi = c * NW + w
                e0, e1 = starts1[gi], starts1[gi + 1]
                n_e = int(e1 - e0)
                cols = wchunks[w]
                k = len(cols)
                sv = np.zeros(k * 128, dtype=np.int64)
                sv[:n_e] = ss1[e0:e1]
                dv = np.full(k * 128, -1.0, dtype=np.float32)
                dv[:n_e] = (ds1[e0:e1] - (c * SLICE + w * 128)).astype(np.float32)
                for j, (gcol, _r) in enumerate(cols):
                    src_cols[gcol] = sv[j * 128 : (j + 1) * 128]
                    dst1_cols[gcol] = dv[j * 128 : (j + 1) * 128]
        x_edges = np.ascontiguousarray(
            xs[src_cols.reshape(-1)].reshape(C1, 128, DIN).transpose(1, 0, 2)
        )  # [128, C1, DIN] bf16
        oh = (
            (dst1_cols[:, :, None] == np.arange(128, dtype=np.float32)[None, None, :])
            .astype(ml_dtypes.float8_e4m3)
            .transpose(1, 0, 2)
        )  # [128, C1, 128] fp8, pure 0/1

        nodes = np.arange(c * SLICE, (c + 1) * SLICE)
        tmp = np.ones(NPAD, dtype=np.float32)
        tmp[:SLICE] = dinv[nodes]
        dinv_col = np.ascontiguousarray(tmp.reshape(NW, 128).T)  # [128, NW] f32

        Bc = np.zeros((NPAD, NG), dtype=np.float32)
        Bc[:SLICE] = B[:, c * SLICE : (c + 1) * SLICE].T
        Bt = np.ascontiguousarray(
            Bc.reshape(NW, 128, NG).transpose(1, 0, 2).reshape(128, NW * NG)
        ).astype(ml_dtypes.bfloat16)  # [128, NW*64]

        per_core.append(
            dict(
                x_edges=x_edges.reshape(128, C1 * DIN),
                oh=np.ascontiguousarray(oh.reshape(128, C1 * 128)),
                dinv_col=dinv_col,
                Bt=Bt,
            )
        )
    return meta, per_core, cnt.astype(np.float32)


def _build_program(meta):
    K1 = np.array(meta)
    b1, C1 = _layout1(K1)

    nc = bacc.Bacc("TRN2", target_bir_lowering=False, debug=False, num_devices=NCORES)

    def din(name, shape, dt=F32):
        return nc.dram_tensor(name, shape, dt, kind="ExternalInput").ap()

    x_edges = din("x_edges", [128, C1 * DIN], FP8)
    oh_in = din("oh", [128, C1 * 128], FP8)
    dinv_col_in = din("dinv_col", [128, NW])
    Bt_in = din("Bt", [128, NW * NG], BF16)
    cnt_in = din("cnt", [NG, 1])
    W1t = din("W1t", [128, 2 * DH], BF16)
    b1c = din("b1c", [128, DH // 128])
    W2t = din("W2t", [128, 4 * (DH // 2)], BF16)
    b2r = din("b2r", [128, DH // 2])
    Wf1 = din("Wf1", [DH // 2, DH // 4])
    bf1c = din("bf1c", [128, 1])
    Wf2 = din("Wf2", [DH // 4, DOUT])
    bf2c = din("bf2c", [DOUT, 1])
    out = nc.dram_tensor("out", [NG, DOUT], F32, kind="ExternalOutput").ap()

    with tile.TileContext(nc) as tc:
        with (
            tc.tile_pool(name="const", bufs=1) as cp,
            tc.tile_pool(name="big", bufs=1) as bigp,
            tc.tile_pool(name="work", bufs=1) as wp,
            tc.tile_pool(name="psum", bufs=1, space="PSUM") as pp,
            tc.tile_pool(name="dram", bufs=1, space="DRAM") as dp,
        ):
            def load(ap_in, shape, dt=F32, pool=cp):
                t = pool.tile(shape, dt, name=ap_in.tensor.name + "_sb")
                nc.sync.dma_start(t[:], ap_in[:])
                return t

            dinv_col = load(dinv_col_in, [128, NW])
            Bt_sb = load(Bt_in, [128, NW * NG], BF16)
            W1_sb = load(W1t, [128, 2 * DH], BF16)
            W2_sb = load(W2t, [128, 4 * (DH // 2)], BF16)
            b1_sb = load(b1c, [128, DH // 128])
            b2_sb = load(b2r, [128, DH // 2])
            bf1_sb = load(bf1c, [128, 1])
            bf2_sb = load(bf2c, [DOUT, 1])
            cnt_sb = load(cnt_in, [NG, 1])
            Wf1_sb = [cp.tile([128, DH // 4], F32, name=f"wf1_{k}") for k in range(2)]
            for k in range(2):
                nc.sync.dma_start(Wf1_sb[k][:], Wf1[k * 128 : (k + 1) * 128, :])
            Wf2_sb = cp.tile([128, DOUT], F32)
            nc.sync.dma_start(Wf2_sb[:], Wf2[:])
            idbf = cp.tile([128, 128], BF16)
            make_identity(nc, idbf[:])
            idf32 = cp.tile([128, 128], F32)
            make_identity(nc, idf32[:])

            h1s = [bigp.tile([128, NPAD], BF16, name=f"h1s_{k}") for k in range(4)]
            sfm_groups: dict = {}

            def sfm_of(g):
                if g not in sfm_groups:
                    sfm_groups[g] = [
                        wp.tile([128, 512], BF16, tag=f"sfm{k}", bufs=2, name=f"sfm{k}_{g}")
                        for k in range(2)
                    ]
                return sfm_groups[g]

            g_local = dp.tile([NG, DH // 2], F32)
            g_all = dp.tile([NCORES * NG, DH // 2], F32, addr_space="Shared")
            pg = pp.tile([NG, DH // 2], F32, tag="pool", bufs=1, name="pg")

            # ---- phase A: conv1 aggregation (one batch = GB windows) ----
            def emit_agg_batch(ws, wchunks, nch):
                c0 = wchunks[ws[0]][0][0]
                G1 = wp.tile([128, nch, DIN], FP8, tag="G1", bufs=2, name=f"g1_{ws[0]}")
                nc.sync.dma_start(
                    G1[:].rearrange("p c d -> p (c d)"),
                    x_edges[:, c0 * DIN : (c0 + nch) * DIN],
                )
                oh1 = wp.tile([128, nch, 128], FP8, tag="oh", bufs=2, name=f"oh1_{ws[0]}")
                nc.sync.dma_start(
                    oh1[:].rearrange("p c d -> p (c d)"),
                    oh_in[:, c0 * 128 : (c0 + nch) * 128],
                )
                for w in ws:
                    cols = wchunks[w]
                    acc = pp.tile([128, DIN], F32, tag="agg", bufs=3, name=f"acc1_{w}")
                    for j, (gcol, grel) in enumerate(cols):
                        nc.tensor.matmul(
                            out=acc[:],
                            lhsT=oh1[:, grel, :],
                            rhs=G1[:, grel, :],
                            start=(j == 0),
                            stop=(j == len(cols) - 1),
                        )
                    snm = wp.tile([128, DIN], BF16, tag="snm", bufs=2, name=f"snm_{w}")
                    nc.scalar.activation(snm[:], acc[:], AF.Copy, scale=dinv_col[:, w : w + 1])
                    sf = sfm_of(w // 4)
                    wc = (w % 4) * 128
                    for k in range(2):
                        pt = pp.tile([128, 128], BF16, tag="t", bufs=2, name=f"pt_{w}_{k}")
                        nc.tensor.transpose(pt[:], snm[:, k * 128 : (k + 1) * 128], idbf[:])
                        if k == 0:
                            nc.scalar.activation(sf[k][:, wc : wc + 128], pt[:], AF.Copy)
                        else:
                            nc.vector.tensor_copy(sf[k][:, wc : wc + 128], pt[:])

            # ---- phase B: dense W1 (feature-major), relu -> h1s ----
            def emit_dense_group(g):
                c0 = g * 512
                cw = min(512, NPAD - c0)
                sf = sfm_of(g)
                for m in range(4):
                    ph = pp.tile([128, 512], F32, tag="h1", bufs=2, name=f"ph1_{g}_{m}")
                    for k in range(2):
                        nc.tensor.matmul(
                            out=ph[:, :cw],
                            lhsT=W1_sb[:, k * DH + m * 128 : k * DH + (m + 1) * 128],
                            rhs=sf[k][:, :cw],
                            start=(k == 0),
                            stop=(k == 1),
                        )
                    nc.scalar.activation(
                        h1s[m][:, c0 : c0 + cw], ph[:, :cw], AF.Relu, bias=b1_sb[:, m : m + 1]
                    )

            # ---- phase C: p = dinv*(h1@W2) node-major; fused pool matmul ----
            def emit_p_chunk(cc):
                c0 = cc * 128
                ppm = pp.tile([128, DH // 2], F32, tag="agg", bufs=3, name=f"pp_{cc}")
                for k in range(4):
                    nc.tensor.matmul(
                        out=ppm[:],
                        lhsT=h1s[k][:, c0 : c0 + 128],
                        rhs=W2_sb[:, k * (DH // 2) : (k + 1) * (DH // 2)],
                        start=(k == 0),
                        stop=(k == 3),
                    )
                pb = wp.tile([128, DH // 2], BF16, tag="pb", bufs=2, name=f"pb_{cc}")
                nc.vector.tensor_scalar_mul(pb[:], ppm[:], dinv_col[:, cc : cc + 1])
                nc.tensor.matmul(
                    out=pg[:],
                    lhsT=Bt_sb[:, cc * NG : (cc + 1) * NG],
                    rhs=pb[:],
                    start=(cc == 0),
                    stop=(cc == NW - 1),
                )

            g_done = 0
            p_done = 0
            for b in range(NB):
                emit_agg_batch(*b1[b])
                wins_done = min((b + 1) * GB, NW)
                while g_done < NGRP and (
                    (g_done + 1) * 4 <= wins_done or wins_done == NW
                ):
                    emit_dense_group(g_done)
                    g_done += 1
                    hi = min(g_done * 4, NW)
                    while p_done < hi:
                        emit_p_chunk(p_done)
                        p_done += 1

            # ---- phase E: AllGather partials (Mesh, cheap) + local sum ----
            gsb = wp.tile([NG, DH // 2], F32)
            nc.vector.tensor_copy(gsb[:], pg[:])
            nc.sync.dma_start(g_local[:], gsb[:])
            nc.gpsimd.collective_compute(
                "AllGather",
                OP.bypass,
                replica_groups=[list(range(NCORES))],
                ins=[g_local.opt()],
                outs=[g_all.opt()],
            )
            # [512, 256] DRAM -> [64, 8, 256] SBUF (graph-partition, rank-major cols)
            parts = wp.tile([NG, NCORES, DH // 2], F32, name="parts")
            nc.sync.dma_start(
                parts[:],
                g_all[:].rearrange("(r p) f -> p r f", r=NCORES),
            )
            s4 = wp.tile([NG, 4, DH // 2], F32, name="s4")
            nc.vector.tensor_tensor(
                out=s4[:], in0=parts[:, 0:4, :], in1=parts[:, 4:8, :], op=OP.add
            )
            s2 = wp.tile([NG, 2, DH // 2], F32, name="s2")
            nc.vector.tensor_tensor(
                out=s2[:], in0=s4[:, 0:2, :], in1=s4[:, 2:4, :], op=OP.add
            )
            gsum = wp.tile([NG, DH // 2], F32)
            nc.vector.tensor_tensor(
                out=gsum[:],
                in0=s2[:, 0, :],
                in1=s2[:, 1, :],
                op=OP.add,
            )
            cinv = wp.tile([NG, 1], F32)
            nc.vector.reciprocal(cinv[:], cnt_sb[:])
            gmean = wp.tile([NG, DH // 2], F32)
            nc.vector.scalar_tensor_tensor(
                out=gmean[:],
                in0=gsum[:],
                scalar=cinv[:, 0:1],
                in1=b2_sb[:NG, :],
                op0=OP.mult,
                op1=OP.add,
            )
            grelu = wp.tile([NG, DH // 2], F32)
            nc.scalar.activation(grelu[:], gmean[:], AF.Relu)

            # ---- phase F: MLP on all 64 graphs (replicated; core 0 wins) ----
            g_fm = [wp.tile([128, NG], F32, name=f"gfm_{k}") for k in range(2)]
            for k in range(2):
                pt = pp.tile([128, NG], F32, tag="t", bufs=2, name=f"gt_{k}")
                nc.tensor.transpose(pt[:], grelu[:, k * 128 : (k + 1) * 128], idf32[:NG, :NG])
                nc.vector.tensor_copy(g_fm[k][:], pt[:])
            pz = pp.tile([128, NG], F32, tag="h1", bufs=2, name="pz")
            for k in range(2):
                nc.tensor.matmul(
                    out=pz[:], lhsT=Wf1_sb[k][:], rhs=g_fm[k][:], start=(k == 0), stop=(k == 1)
                )
            zsb = wp.tile([128, NG], F32)
            nc.scalar.activation(zsb[:], pz[:], AF.Relu, bias=bf1_sb[:, 0:1])
            po = pp.tile([DOUT, NG], F32, tag="t", bufs=2, name="po")
            nc.tensor.matmul(out=po[:], lhsT=Wf2_sb[:], rhs=zsb[:], start=True, stop=True)
            osb = wp.tile([DOUT, NG], F32)
            nc.scalar.activation(osb[:], po[:], AF.Relu, bias=bf2_sb[:, 0:1])
            pout = pp.tile([NG, DOUT], F32, tag="t", bufs=2, name="pout")
            nc.tensor.transpose(pout[:], osb[:], idf32[:DOUT, :DOUT])
            out_sb = wp.tile([NG, DOUT], F32)
            nc.vector.tensor_copy(out_sb[:], pout[:])
            nc.sync.dma_start(out[:], out_sb[:])

    nc.compile()
    return nc


def _get_program(meta):
    if meta not in _COMPILED:
        _COMPILED[meta] = _build_program(meta)
    return _COMPILED[meta]


def _make_in_maps(W1, b1, W2, b2, Wf1, bf1, Wf2, bf2, per_core, cnt):
    W1t = np.concatenate(
        [np.asarray(W1, np.float32)[k * 128 : (k + 1) * 128, :] for k in range(2)], axis=1
    ).astype(ml_dtypes.bfloat16)
    W2t = np.concatenate(
        [np.asarray(W2, np.float32)[k * 128 : (k + 1) * 128, :] for k in range(4)], axis=1
    ).astype(ml_dtypes.bfloat16)
    shared = dict(
        cnt=np.asarray(cnt, np.float32).reshape(NG, 1),
        W1t=W1t,
        b1c=np.ascontiguousarray(np.asarray(b1, np.float32).reshape(DH // 128, 128).T),
        W2t=W2t,
        b2r=np.ascontiguousarray(np.tile(np.asarray(b2, np.float32)[None, :], (128, 1))),
        Wf1=np.asarray(Wf1, np.float32),
        bf1c=np.asarray(bf1, np.float32).reshape(DH // 4, 1),
        Wf2=np.asarray(Wf2, np.float32),
        bf2c=np.asarray(bf2, np.float32).reshape(DOUT, 1),
    )
    return [dict(shared, **per_core[c]) for c in range(NCORES)]


def kernel(
    x, W1, b1, W2, b2, Wf1, bf1, Wf2, bf2, edge_index, batch, num_graphs, _trace=False
):
    assert int(num_graphs) == NG
    meta, per_core, cnt = _preprocess(
        np.asarray(x), np.asarray(edge_index), np.asarray(batch)
    )
    nc = _get_program(meta)
    in_maps = _make_in_maps(W1, b1, W2, b2, Wf1, bf1, Wf2, bf2, per_core, cnt)
    res = bass_utils.run_bass_kernel_spmd(
        nc, in_maps, core_ids=list(range(NCORES)), trace=_trace
    )
    out = np.asarray(res.results[0]["out"], np.float32)
    if _trace:
        kernel._last_results = res
    return out
